# revision 1
# baseline (speedup 1.0000x reference)
"""MetabolicPathwayLoss Trainium2 kernel (8-core SPMD).

Loss =  mean((X X^T - Yn Yn^T)^2)            [coherence]
      + mean((X - A X)^2)                    [structure]
      + mean((X - W)^2)                      [weight]
with X = pathway_predictions [N,P], Yn = row-normalized node_embeddings [N,D],
A = pathway_adjacency [N,N], W = pathway_weights [N,P]; N=8192, P=128, D=256.

Strategy
--------
The O(N^2) similarity matrices are never materialized:
    mean((X X^T - Yn Yn^T)^2) = (||X^T X||_F^2 - 2||X^T Yn||_F^2 + ||Yn^T Yn||_F^2)/N^2
so the coherence term reduces to three tiny Gram matrices ([P,P], [P,D], [D,D]).
The structure term uses (X - A X) = -(A - I) X, with the identity folded into
the adjacency on the host, so the device computes one [N,N]x[N,P] GEMM streamed
straight out of HBM and square-reduces the PSUM output.

Sharding: adjacency rows are sharded across the 8 cores. Core c computes
T_c^T = X^T (A'-shard_c)^T with stationary X tiles and the (host-pre-transposed,
fp16-cast) adjacency shard streamed as the moving operand, plus partial Gram
matrices over its row shard. The host sums the per-core partials (the "scalar
all-reduce") in float64 and assembles the final scalar.

fp16 on device: inputs are cast to fp16 on the host (A in [0,1], X/W/Y ~ N(0,1),
Yn in [-1,1] - all comfortably in fp16 range), matmul accumulation is fp32 in
PSUM, reductions are fp32, final combine is float64. Validated end-to-end
relative error vs a float64 reference: ~4e-6.
"""

import numpy as np

N, P, D, CORES = 8192, 128, 256, 8
R = N // CORES  # adjacency rows per core
NT = R // 512  # 512-column output tiles per core (2)
KC = N // 128  # contraction chunks (64)
SH = R // 128  # shard row chunks per core (8)
COS_EPS = 1e-8

# output staging layout (fp32, [128, OUTW])
G1_OFF = 0  # [128, 128]   X_c^T X_c
M_OFF = 128  # [128, 256]   X_c^T Yn_c
G2A_OFF = 384  # [128, 256]   Yn_c[:, :128]^T Yn_c
G2B_OFF = 640  # [128, 256]   Yn_c[:, 128:]^T Yn_c
ST_OFF = 896  # [128, NT]    sum((A'X)^2) partials
WT_OFF = ST_OFF + NT  # [128, SH]    sum((X-W)^2) partials
OUTW = WT_OFF + SH

_PROGRAM = None


def _build_program(repeats=1, grp=8, adj_bufs=3, alt_rings=False):
    # repeats>1 re-runs the full kernel body inside one NEFF; used by
    # timeit_hw.py to measure steady-state per-iteration HW time by slope.
    # grp/adj_bufs/alt_rings are perf-tuning experiment knobs.
    import concourse.mybir as mybir
    import concourse.tile as tile
    from concourse import bacc

    f16 = mybir.dt.float16
    f32 = mybir.dt.float32

    # Bacc (not raw Bass): its compile() pass legalizes per-instruction sync
    # waits, which walrus codegen limits per ISA struct.
    nc = bacc.Bacc("TRN2", target_bir_lowering=False, debug=False)

    adjT = nc.dram_tensor("adjt", [N, R], f16, kind="ExternalInput").ap()
    x = nc.dram_tensor("x", [N, P], f16, kind="ExternalInput").ap()
    # xw packs this core's X rows (cols 0:P) and W rows (cols P:2P) so one DMA
    # (one sem lane) feeds the (x-w) DVE op — the TT ISA slot fits one wait.
    xw = nc.dram_tensor("xw", [R, 2 * P], f16, kind="ExternalInput").ap()
    y = nc.dram_tensor("y", [R, D], f16, kind="ExternalInput").ap()
    out = nc.dram_tensor("out", [128, OUTW], f32, kind="ExternalOutput").ap()

    GRP = grp  # adjacency k-chunks per DMA (grp=8 -> 2 MiB per load)

    with tile.TileContext(nc) as tc:
        with (
            tc.tile_pool(name="const", bufs=1) as const,
            tc.tile_pool(name="adj", bufs=adj_bufs) as adjp,
            tc.tile_pool(name="tmp", bufs=2) as tmp,
            tc.tile_pool(name="ps", bufs=1, space="PSUM") as ps,
        ):
          for _rep in range(repeats):
              # resident inputs (small loads on the ACT HWDGE ring so they do
              # not queue behind the adjacency stream on the SP ring)
              x_sb = const.tile([128, KC, P], f16)
              nc.scalar.dma_start(x_sb[:], x.rearrange("(t p) d -> p t d", p=128))
              xw_sb = const.tile([128, SH, 2 * P], f16)
              nc.scalar.dma_start(xw_sb[:], xw.rearrange("(t p) d -> p t d", p=128))
              y_sb = const.tile([128, SH, D], f16)
              nc.scalar.dma_start(y_sb[:], y.rearrange("(t p) d -> p t d", p=128))

              stage = const.tile([128, OUTW], f32)

              # ---- structure GEMM: T' = X^T A'^T, accumulated over all 64
              # k-chunks into NT psum banks; adjacency streams from HBM
              t_ps = []
              for i in range(NT):
                  tp = ps.tile([128, 512], f32, tag=f"t{i}", name=f"t_ps{i}")
                  t_ps.append(tp)
              for g in range(KC // GRP):
                  a_sb = adjp.tile([128, GRP, R], f16)
                  ring = nc.scalar if (alt_rings and g % 2) else nc.sync
                  ring.dma_start(
                      a_sb[:],
                      adjT[g * GRP * 128 : (g + 1) * GRP * 128, :].rearrange(
                          "(t p) n -> p t n", p=128
                      ),
                  )
                  for t in range(GRP):
                      k = g * GRP + t
                      for i in range(NT):
                          nc.tensor.matmul(
                              t_ps[i][:],
                              x_sb[:, k, :],
                              a_sb[:, t, i * 512 : (i + 1) * 512],
                              start=(k == 0),
                              stop=(k == KC - 1),
                          )

              # ---- embedding row norms and normalization (fp32 math)
              ss = const.tile([128, SH], f32)
              for i in range(SH):
                  sq = tmp.tile([128, D], f32, tag="sq", name=f"sq{i}")
                  nc.scalar.activation(
                      sq[:],
                      y_sb[:, i, :],
                      mybir.ActivationFunctionType.Square,
                      accum_out=ss[:, i : i + 1],
                  )
              nrm = const.tile([128, SH], f32)
              nc.scalar.sqrt(nrm[:], ss[:])
              nc.vector.tensor_scalar_max(nrm[:], nrm[:], COS_EPS)
              inv = const.tile([128, SH], f32)
              nc.vector.reciprocal(inv[:], nrm[:])
              yn_sb = const.tile([128, SH, D], f16)
              for i in range(SH):
                  nc.vector.tensor_scalar_mul(
                      yn_sb[:, i, :], y_sb[:, i, :], inv[:, i : i + 1]
                  )

              # ---- Gram matrices over this core's row shard
              g1_ps = ps.tile([128, P], f32, tag="g1")
              m_ps = ps.tile([128, D], f32, tag="m")
              g2a_ps = ps.tile([128, D], f32, tag="g2a")
              g2b_ps = ps.tile([128, D], f32, tag="g2b")
              for i in range(SH):
                  s, e = (i == 0), (i == SH - 1)
                  nc.tensor.matmul(
                      g1_ps[:], xw_sb[:, i, 0:P], xw_sb[:, i, 0:P], start=s, stop=e
                  )
                  nc.tensor.matmul(
                      m_ps[:], xw_sb[:, i, 0:P], yn_sb[:, i, :], start=s, stop=e
                  )
                  nc.tensor.matmul(
                      g2a_ps[:], yn_sb[:, i, 0:128], yn_sb[:, i, :], start=s, stop=e
                  )
                  nc.tensor.matmul(
                      g2b_ps[:], yn_sb[:, i, 128:256], yn_sb[:, i, :], start=s, stop=e
                  )

              # ---- epilogues: square-reduce T' psum (ACT: one PSUM input only);
              # (x-w)^2 partials
              for i in range(NT):
                  scr = tmp.tile([128, 512], f32, tag="scr", name=f"scr{i}")
                  nc.scalar.activation(
                      scr[:],
                      t_ps[i][:],
                      mybir.ActivationFunctionType.Square,
                      accum_out=stage[:, ST_OFF + i : ST_OFF + i + 1],
                  )
              for i in range(SH):
                  dif = tmp.tile([128, P], f32, tag="dif", name=f"dif{i}")
                  nc.vector.tensor_sub(dif[:], xw_sb[:, i, 0:P], xw_sb[:, i, P : 2 * P])
                  sd = tmp.tile([128, P], f32, tag="sd", name=f"sd{i}")
                  nc.scalar.activation(
                      sd[:],
                      dif[:],
                      mybir.ActivationFunctionType.Square,
                      accum_out=stage[:, WT_OFF + i : WT_OFF + i + 1],
                  )

              nc.scalar.copy(stage[:, G1_OFF : G1_OFF + P], g1_ps[:])
              nc.scalar.copy(stage[:, M_OFF : M_OFF + D], m_ps[:])
              nc.scalar.copy(stage[:, G2A_OFF : G2A_OFF + D], g2a_ps[:])
              nc.scalar.copy(stage[:, G2B_OFF : G2B_OFF + D], g2b_ps[:])

              nc.scalar.dma_start(out, stage[:])

    nc.compile()
    return nc


def _get_program():
    global _PROGRAM
    if _PROGRAM is None:
        _PROGRAM = _build_program()
    return _PROGRAM


def _prep_inputs(pathway_predictions, node_embeddings, pathway_adjacency, pathway_weights):
    f16 = np.float16
    x16 = np.ascontiguousarray(pathway_predictions, dtype=np.float32).astype(f16)
    y16 = np.ascontiguousarray(node_embeddings, dtype=np.float32).astype(f16)
    w16 = np.ascontiguousarray(pathway_weights, dtype=np.float32).astype(f16)
    A = np.asarray(pathway_adjacency)

    in_maps = []
    for c in range(CORES):
        r0 = c * R
        # transposed shard: adjt[k, j] = A[r0 + j, k]; identity folded in
        adjt = np.ascontiguousarray(A[r0 : r0 + R, :].T).astype(f16)
        j = np.arange(R)
        adjt[r0 + j, j] = (A[r0 + j, r0 + j].astype(np.float64) - 1.0).astype(f16)
        in_maps.append(
            {
                "adjt": adjt,
                "x": x16,
                "xw": np.ascontiguousarray(
                    np.concatenate([x16[r0 : r0 + R], w16[r0 : r0 + R]], axis=1)
                ),
                "y": y16[r0 : r0 + R],
            }
        )
    return in_maps


def _combine(outs):
    f64 = np.float64
    g1 = np.zeros((P, P), f64)
    m = np.zeros((P, D), f64)
    g2 = np.zeros((D, D), f64)
    st = f64(0.0)
    wt = f64(0.0)
    for o in outs:
        o = o.astype(f64)
        g1 += o[:, G1_OFF : G1_OFF + P]
        m += o[:, M_OFF : M_OFF + D]
        g2[0:128] += o[:, G2A_OFF : G2A_OFF + D]
        g2[128:256] += o[:, G2B_OFF : G2B_OFF + D]
        st += o[:, ST_OFF : ST_OFF + NT].sum()
        wt += o[:, WT_OFF : WT_OFF + SH].sum()
    coherence = ((g1 * g1).sum() - 2.0 * (m * m).sum() + (g2 * g2).sum()) / (
        f64(N) * f64(N)
    )
    structure = st / (f64(N) * f64(P))
    weight = wt / (f64(N) * f64(P))
    return np.asarray(coherence + structure + weight, dtype=np.float32)


def kernel(pathway_predictions, node_embeddings, pathway_adjacency, pathway_weights):
    from concourse.bass_utils import run_bass_kernel_spmd

    nc = _get_program()
    in_maps = _prep_inputs(
        pathway_predictions, node_embeddings, pathway_adjacency, pathway_weights
    )
    res = run_bass_kernel_spmd(nc, in_maps, list(range(CORES)))
    return _combine([r["out"] for r in res.results])



# revision 2
# speedup vs baseline: 1.7914x; 1.7914x over previous
"""MetabolicPathwayLoss Trainium2 kernel (8-core SPMD), fp8 streaming version.

Loss =  mean((X X^T - Yn Yn^T)^2)            [coherence]
      + mean((X - A X)^2)                    [structure]
      + mean((X - W)^2)                      [weight]
with X = pathway_predictions [N,P], Yn = row-normalized node_embeddings [N,D],
A = pathway_adjacency [N,N], W = pathway_weights [N,P]; N=8192, P=128, D=256.

Strategy
--------
The O(N^2) similarity matrices are never materialized:
    mean((X X^T - Yn Yn^T)^2) = (||X^T X||_F^2 - 2||X^T Yn||_F^2 + ||Yn^T Yn||_F^2)/N^2
so the coherence term reduces to three tiny Gram matrices ([P,P], [P,D], [D,D]).
The structure term uses (X - A X) = -(A - I) X, identity folded into A on the
host; the device computes one [N,N]x[N,P] GEMM streamed from HBM and
square-reduces the PSUM output.

v2 changes vs the fp16 baseline:
 - The adjacency stream and the stationary X are cast to fp8 e4m3 on the host,
   halving the dominant HBM traffic, and the structure GEMM runs with
   perf_mode=DoubleRow (2 fp8 k-rows per PE cell per cycle).
 - All inputs are host-packed into the exact SBUF layout ([128, ...] with
   contiguous per-partition lines), so each DMA is a few large descriptors
   instead of thousands of 256B row gathers (the fp16 baseline spent ~25us of
   scalar-engine descriptor generation before the first matmul could start,
   which also HAM-throttled the PE).
 - fp8 quantization bias on the structure term is corrected on the host from
   O(N^2)-elementwise statistics (column sums/squares of A and the
   quantization residuals); validated to reduce the structure-term error from
   ~-14.3 absolute to ~+1 on seed-0 data.

Sharding: adjacency rows are sharded across the 8 cores; each core computes
T_c = A'_c X via stationary X k-chunks with the fp8 adjacency shard streamed
as the moving operand, plus partial Gram matrices over its row shard. The host
sums per-core partials (the "scalar all-reduce") in float64.
"""

import numpy as np

N, P, D, CORES = 8192, 128, 256, 8
R = N // CORES  # adjacency rows per core
NT = R // 512  # 512-column output tiles per core (2)
KC = N // 128  # contraction chunks (64)
SH = R // 128  # shard row chunks per core (8)
GRP = 8  # adjacency k-chunks per DMA group (1 MiB fp8)
COS_EPS = 1e-8

# output staging layout (fp32, [128, OUTW])
G1_OFF = 0  # [128, 128]   X_c^T X_c
M_OFF = 128  # [128, 256]   X_c^T Yn_c
G2A_OFF = 384  # [128, 256]   Yn_c[:, :128]^T Yn_c
G2B_OFF = 640  # [128, 256]   Yn_c[:, 128:]^T Yn_c
ST_OFF = 896  # [128, NT]    sum((A'X)^2) partials
WT_OFF = ST_OFF + NT  # [128, SH]    sum((X-W)^2) partials
OUTW = WT_OFF + SH

_PROGRAM = None


def _build_program():
    import concourse.mybir as mybir
    import concourse.tile as tile
    from concourse import bacc

    f8 = mybir.dt.float8e4
    f16 = mybir.dt.float16
    f32 = mybir.dt.float32
    DR = mybir.MatmulPerfMode.DoubleRow

    nc = bacc.Bacc("TRN2", target_bir_lowering=False, debug=False)

    # all inputs pre-packed on the host into SBUF layout: [128 partitions, cols]
    # with contiguous per-partition lines.
    adj = nc.dram_tensor("adj", [128, KC * R], f8, kind="ExternalInput").ap()
    x = nc.dram_tensor("x", [128, KC * P], f8, kind="ExternalInput").ap()
    xw = nc.dram_tensor("xw", [128, SH * 2 * P], f16, kind="ExternalInput").ap()
    y = nc.dram_tensor("y", [128, SH * D], f16, kind="ExternalInput").ap()
    out = nc.dram_tensor("out", [128, OUTW], f32, kind="ExternalOutput").ap()

    NG = KC // GRP  # DMA groups (8)
    PAIRS = GRP // 2  # DoubleRow k-chunk pairs per group (4)

    with tile.TileContext(nc) as tc:
        with (
            tc.tile_pool(name="const", bufs=1) as const,
            tc.tile_pool(name="adj", bufs=3) as adjp,
            tc.tile_pool(name="tmp", bufs=2) as tmp,
            tc.tile_pool(name="ps", bufs=1, space="PSUM") as ps,
        ):
            # resident inputs on the ACT HWDGE ring; adjacency streams on SP.
            x_sb = const.tile([128, KC, P], f8)
            nc.scalar.dma_start(x_sb[:], x.rearrange("p (k d) -> p k d", k=KC))
            xw_sb = const.tile([128, SH, 2 * P], f16)
            nc.scalar.dma_start(xw_sb[:], xw.rearrange("p (t d) -> p t d", t=SH))
            y_sb = const.tile([128, SH, D], f16)
            nc.scalar.dma_start(y_sb[:], y.rearrange("p (t d) -> p t d", t=SH))

            stage = const.tile([128, OUTW], f32)

            # ---- embedding row norms and normalization (fp32 math)
            ss = const.tile([128, SH], f32)
            for i in range(SH):
                sq = tmp.tile([128, D], f32, tag="sq", name=f"sq{i}")
                nc.scalar.activation(
                    sq[:],
                    y_sb[:, i, :],
                    mybir.ActivationFunctionType.Square,
                    accum_out=ss[:, i : i + 1],
                )
            nrm = const.tile([128, SH], f32)
            nc.scalar.sqrt(nrm[:], ss[:])
            nc.vector.tensor_scalar_max(nrm[:], nrm[:], COS_EPS)
            inv = const.tile([128, SH], f32)
            nc.vector.reciprocal(inv[:], nrm[:])
            yn_sb = const.tile([128, SH, D], f16)
            for i in range(SH):
                nc.vector.tensor_scalar_mul(
                    yn_sb[:, i, :], y_sb[:, i, :], inv[:, i : i + 1]
                )

            # ---- PSUM tiles
            t_ps = [
                ps.tile([128, 512], f32, tag=f"t{i}", name=f"t_ps{i}")
                for i in range(NT)
            ]
            g1_ps = ps.tile([128, P], f32, tag="g1")
            m_ps = ps.tile([128, D], f32, tag="m")
            g2a_ps = ps.tile([128, D], f32, tag="g2a")
            g2b_ps = ps.tile([128, D], f32, tag="g2b")

            # ---- structure GEMM (fp8 DoubleRow), Gram matmuls interleaved so
            # the PE tail after the last adjacency group stays short
            for g in range(NG):
                a_sb = adjp.tile([128, GRP, R], f8)
                nc.sync.dma_start(
                    a_sb[:],
                    adj[:, g * GRP * R : (g + 1) * GRP * R].rearrange(
                        "p (t n) -> p t n", t=GRP
                    ),
                )
                for t in range(PAIRS):
                    pair = g * PAIRS + t
                    for i in range(NT):
                        nc.tensor.matmul(
                            t_ps[i][:],
                            x_sb[:, g * GRP + 2 * t : g * GRP + 2 * t + 2, :],
                            a_sb[:, 2 * t : 2 * t + 2, i * 512 : (i + 1) * 512],
                            start=(pair == 0),
                            stop=(pair == KC // 2 - 1),
                            perf_mode=DR,
                        )
                # one Gram chunk per adjacency group, two groups in (norms done)
                if g >= 2:
                    i = g - 2
                    s, e = (i == 0), False
                    nc.tensor.matmul(
                        g1_ps[:], xw_sb[:, i, 0:P], xw_sb[:, i, 0:P], start=s, stop=e
                    )
                    nc.tensor.matmul(
                        m_ps[:], xw_sb[:, i, 0:P], yn_sb[:, i, :], start=s, stop=e
                    )
                    nc.tensor.matmul(
                        g2a_ps[:], yn_sb[:, i, 0:128], yn_sb[:, i, :], start=s, stop=e
                    )
                    nc.tensor.matmul(
                        g2b_ps[:], yn_sb[:, i, 128:256], yn_sb[:, i, :], start=s, stop=e
                    )
            for i in range(NG - 2, SH):
                s, e = False, (i == SH - 1)
                nc.tensor.matmul(
                    g1_ps[:], xw_sb[:, i, 0:P], xw_sb[:, i, 0:P], start=s, stop=e
                )
                nc.tensor.matmul(
                    m_ps[:], xw_sb[:, i, 0:P], yn_sb[:, i, :], start=s, stop=e
                )
                nc.tensor.matmul(
                    g2a_ps[:], yn_sb[:, i, 0:128], yn_sb[:, i, :], start=s, stop=e
                )
                nc.tensor.matmul(
                    g2b_ps[:], yn_sb[:, i, 128:256], yn_sb[:, i, :], start=s, stop=e
                )

            # ---- (x-w)^2 partials (runs on ACT/DVE during the stream)
            for i in range(SH):
                dif = tmp.tile([128, P], f32, tag="dif", name=f"dif{i}")
                nc.vector.tensor_sub(dif[:], xw_sb[:, i, 0:P], xw_sb[:, i, P : 2 * P])
                sd = tmp.tile([128, P], f32, tag="sd", name=f"sd{i}")
                nc.scalar.activation(
                    sd[:],
                    dif[:],
                    mybir.ActivationFunctionType.Square,
                    accum_out=stage[:, WT_OFF + i : WT_OFF + i + 1],
                )

            # Gram psum -> stage, shipped out mid-kernel on the ACT ring
            nc.scalar.copy(stage[:, G1_OFF : G1_OFF + P], g1_ps[:])
            nc.scalar.copy(stage[:, M_OFF : M_OFF + D], m_ps[:])
            nc.scalar.copy(stage[:, G2A_OFF : G2A_OFF + D], g2a_ps[:])
            nc.scalar.copy(stage[:, G2B_OFF : G2B_OFF + D], g2b_ps[:])
            nc.scalar.dma_start(out[:, 0:ST_OFF], stage[:, 0:ST_OFF])

            # ---- structure epilogue: square-reduce T' psum, then small DMA
            for i in range(NT):
                scr = tmp.tile([128, 512], f32, tag="scr", name=f"scr{i}")
                nc.scalar.activation(
                    scr[:],
                    t_ps[i][:],
                    mybir.ActivationFunctionType.Square,
                    accum_out=stage[:, ST_OFF + i : ST_OFF + i + 1],
                )
            nc.scalar.dma_start(out[:, ST_OFF:OUTW], stage[:, ST_OFF:OUTW])

    nc.compile()
    return nc


def _get_program():
    global _PROGRAM
    if _PROGRAM is None:
        _PROGRAM = _build_program()
    return _PROGRAM


def _pack128(a, chunks):
    """[chunks*128, cols] row-major -> [128, chunks*cols] with row t*128+p on
    partition p at free offset t*cols (the SBUF layout a [128, chunks, cols]
    tile expects, contiguous per partition)."""
    rows, cols = a.shape
    return (
        a.reshape(chunks, 128, cols).transpose(1, 0, 2).reshape(128, chunks * cols)
    )


def _prep_inputs(pathway_predictions, node_embeddings, pathway_adjacency, pathway_weights):
    import ml_dtypes

    e4 = ml_dtypes.float8_e4m3
    f16 = np.float16
    X = np.ascontiguousarray(pathway_predictions, dtype=np.float32)
    Y = np.ascontiguousarray(node_embeddings, dtype=np.float32)
    W = np.ascontiguousarray(pathway_weights, dtype=np.float32)
    A = np.asarray(pathway_adjacency)

    x16, y16, w16 = X.astype(f16), Y.astype(f16), W.astype(f16)
    X8 = X.astype(e4)
    x8_packed = np.ascontiguousarray(_pack128(X8, KC))

    # ---- fp8 bias-correction statistics (O(N^2) elementwise only) ----
    f64 = np.float64
    Xd = X.astype(f64)
    dX = X8.astype(f64) - Xd
    diag = np.diagonal(A).astype(f64)
    r = A.sum(axis=0, dtype=f64) - 1.0  # colsums of A' = A - I
    colsq = np.einsum("ij,ij->j", A, A, dtype=f64) + 1.0 - 2.0 * diag  # colsums A'^2
    v = colsq - r * r / N  # col variance sums
    rdX = r @ dX  # [P]
    rX = r @ Xd  # [P]
    bias1 = 2.0 / (f64(N) * N * P) * (rdX * rX).sum()
    c_xx = (N * ((rdX / N) ** 2).sum() + (v[:, None] * dX * dX).sum()) / (f64(N) * P)
    rowsq_X = (Xd * Xd).sum(axis=1)  # [N]

    in_maps = []
    qsq = np.zeros(N, f64)  # colsums of dA^2, accumulated over shards
    for c in range(CORES):
        r0 = c * R
        # transposed shard: adjt[k, j] = A'[r0 + j, k]
        adjt = np.ascontiguousarray(A[r0 : r0 + R, :].T, dtype=np.float32)
        j = np.arange(R)
        adjt[r0 + j, j] -= 1.0
        adj8 = adjt.astype(e4)
        dAt = adj8.astype(np.float32) - adjt
        qsq += np.einsum("kj,kj->k", dAt, dAt, dtype=f64)
        in_maps.append(
            {
                "adj": np.ascontiguousarray(_pack128(adj8, KC)),
                "x": x8_packed,
                "xw": np.ascontiguousarray(
                    _pack128(np.concatenate([x16[r0 : r0 + R], w16[r0 : r0 + R]], axis=1), SH)
                ),
                "y": np.ascontiguousarray(_pack128(y16[r0 : r0 + R], SH)),
            }
        )
    c_aa = (qsq * rowsq_X).sum() / (f64(N) * P)
    corr = {"st_corr": bias1 + c_xx + c_aa}
    return in_maps, corr


def _combine(outs, corr):
    f64 = np.float64
    g1 = np.zeros((P, P), f64)
    m = np.zeros((P, D), f64)
    g2 = np.zeros((D, D), f64)
    st = f64(0.0)
    wt = f64(0.0)
    for o in outs:
        o = o.astype(f64)
        g1 += o[:, G1_OFF : G1_OFF + P]
        m += o[:, M_OFF : M_OFF + D]
        g2[0:128] += o[:, G2A_OFF : G2A_OFF + D]
        g2[128:256] += o[:, G2B_OFF : G2B_OFF + D]
        st += o[:, ST_OFF : ST_OFF + NT].sum()
        wt += o[:, WT_OFF : WT_OFF + SH].sum()
    coherence = ((g1 * g1).sum() - 2.0 * (m * m).sum() + (g2 * g2).sum()) / (
        f64(N) * f64(N)
    )
    structure = st / (f64(N) * f64(P)) - corr["st_corr"]
    weight = wt / (f64(N) * f64(P))
    return np.asarray(coherence + structure + weight, dtype=np.float32)


def kernel(pathway_predictions, node_embeddings, pathway_adjacency, pathway_weights):
    from concourse.bass_utils import run_bass_kernel_spmd

    nc = _get_program()
    in_maps, corr = _prep_inputs(
        pathway_predictions, node_embeddings, pathway_adjacency, pathway_weights
    )
    res = run_bass_kernel_spmd(nc, in_maps, list(range(CORES)))
    return _combine([r["out"] for r in res.results], corr)
